# revision 1
# baseline (speedup 1.0000x reference)
"""Multi-head attention (b=2, n=2048, dim=1024, h=16, dh=64) on 8 TRN2 NeuronCores.

Sharding: 32 (batch, head) pairs -> 8 cores x (1 batch, 4 heads). No collectives.
Per core:
  inputs : xT  [128, 8*2048] bf16 (x[b].T packed partition-major to match the
                                   SBUF layout: element (p, kt, n) = x[b].T[kt*128+p, n])
           wq  [1024, 256]  bf16  (q-columns of w_qkv for this core's 4 heads, pre-scaled by 1/8)
           wk  [1024, 256]  bf16
           wv  [1024, 256]  bf16
  output : out [4*65, 2048] f32   (per local head: rows 0-63 = unnormalized (attn@v)^T,
                                   row 64 = softmax denominator per query)
Host divides by the denominator and transposes back to [b, n, h*dh].

Device pipeline per core:
  qT/kT = (w.T @ x.T) in [d, n] layout, head-pairs packed 2x64 on partitions (bf16)
  V     = (x @ wv)    in [n, d] layout with a ones column appended (bf16)
  per head pair, per 512-wide query chunk, per 128-wide key block:
    S^T[j,i] = kT.T @ qT   (two K=64 matmuls packed into PE row-groups 0-63 / 64-127)
    A^T      = exp(S^T)    (one ACT instr over both heads' PSUM banks, f32 -> bf16)
    O^T     += [V|1].T @ A^T  (PSUM-accumulated over key blocks; row 64 = rowsum)
"""

import numpy as np
import ml_dtypes

B, N, DIM = 2, 2048, 1024
HEADS, DH = 16, 64
P = 128
KT = DIM // P          # 8 k-tiles
NT = N // P            # 16 n/j blocks
NCH = N // 512         # 4 chunks of 512
HL = 4                 # local heads per core
OROWS = HL * (DH + 1)  # 260 output rows per core

_CACHE = {}
LAST_RESULTS = None
TRACE = False


def _build_nc():
    from contextlib import ExitStack

    import concourse.bass as bass
    import concourse.tile as tile
    from concourse import bacc, mybir

    bf16 = mybir.dt.bfloat16
    f32 = mybir.dt.float32

    nc = bacc.Bacc("TRN2", target_bir_lowering=False)

    xT_d = nc.dram_tensor("xT", [P, KT * N], bf16, kind="ExternalInput")
    wq_d = nc.dram_tensor("wq", [DIM, HL * DH], bf16, kind="ExternalInput")
    wk_d = nc.dram_tensor("wk", [DIM, HL * DH], bf16, kind="ExternalInput")
    wv_d = nc.dram_tensor("wv", [DIM, HL * DH], bf16, kind="ExternalInput")
    out_d = nc.dram_tensor("out", [OROWS, N], f32, kind="ExternalOutput")

    # out rows viewed as [row-within-head, head, n] for packed output DMAs
    out_r = out_d[:, :].rearrange("(hh r) n -> r hh n", r=DH + 1)
    xT_r = xT_d[:, :].rearrange("p (kt n) -> p kt n", kt=KT)
    wq_r = wq_d[:, :].rearrange("(kt p) c -> p kt c", p=P)
    wk_r = wk_d[:, :].rearrange("(kt p) c -> p kt c", p=P)
    wv_r = wv_d[:, :].rearrange("(kt p) c -> p kt c", p=P)

    with tile.TileContext(nc) as tc, ExitStack() as ctx:
        sing = ctx.enter_context(tc.tile_pool(name="sing", bufs=1))
        spool = ctx.enter_context(
            tc.tile_pool(name="s_ps", bufs=3, space=bass.MemorySpace.PSUM)
        )
        opool = ctx.enter_context(
            tc.tile_pool(name="o_ps", bufs=1, space=bass.MemorySpace.PSUM)
        )
        apool = ctx.enter_context(tc.tile_pool(name="a_sb", bufs=14))
        copool = ctx.enter_context(tc.tile_pool(name="o_sb", bufs=4))

        # persistent SBUF tensors
        xT = sing.tile([P, KT, N], bf16, tag="xT")
        wq = sing.tile([P, KT, HL * DH], bf16, tag="wq")
        wk = sing.tile([P, KT, HL * DH], bf16, tag="wk")
        wv = sing.tile([P, KT, HL * DH], bf16, tag="wv")
        # head-pair packed projections: partitions 0-63 head A dims, 64-127 head B
        qT = [sing.tile([P, N], bf16, tag=f"qT{i}", name=f"qT{i}") for i in range(2)]
        kT = [sing.tile([P, N], bf16, tag=f"kT{i}", name=f"kT{i}") for i in range(2)]
        # V in [j, d] layout per j-block per head, with ones column at d=64
        v = sing.tile([P, NT, HL, DH + 1], bf16, tag="v")

        # input DMAs
        nc.gpsimd.dma_start(out=wk[:], in_=wk_r[:])
        nc.gpsimd.dma_start(out=wq[:], in_=wq_r[:])
        nc.gpsimd.dma_start(out=wv[:], in_=wv_r[:])
        # split the x transfer across both HWDGE rings (SP + ACT); the
        # Scalar engine is idle this early so its trigger cost is free
        # flatten each chunk to a 2D [128, 4096] AP so the per-partition 8KB
        # contiguous run is explicit (3D APs were split into small packets)
        xT_f = xT[:].rearrange("p kt n -> p (kt n)")
        for c in range(4):
            eng = nc.sync if c % 2 == 0 else nc.scalar
            eng.dma_start(
                out=xT_f[:, c * 4096 : (c + 1) * 4096],
                in_=xT_d[:, c * 4096 : (c + 1) * 4096],
            )

        # ---- projections ----
        # k, q: out[c, n] = w[:, c].T @ xT.  hp0 upfront; hp1 woven into
        # attention-hp0's periods (PE fills slack while ACT runs exp).
        def proj_unit(wt, dst, hp, nch):
            """Emit the 8 K-accumulated matmuls + copy for one 512-col chunk,
            returned as two 4-matmul halves so weaving stays fine-grained."""
            state = {}

            def half(h):
                if h == 0:
                    state["ps"] = spool.tile([P, 512], f32, tag="sp", name="ps")
                ps = state["ps"]
                for kt in range(4 * h, 4 * h + 4):
                    nc.tensor.matmul(
                        ps[:],
                        wt[:, kt, hp * P : (hp + 1) * P],
                        xT[:, kt, nch * 512 : (nch + 1) * 512],
                        start=(kt == 0),
                        stop=(kt == KT - 1),
                    )
                if h == 1:
                    nc.vector.tensor_copy(dst[:, nch * 512 : (nch + 1) * 512], ps[:])

            return [lambda: half(0), lambda: half(1)]

        # ones column of V (softmax denominator comes out of the PV matmul)
        nc.vector.memset(v[:, :, :, DH : DH + 1], 1.0)
        for unit in [proj_unit(wk, kT[0], 0, 0), proj_unit(wq, qT[0], 0, 0)]:
            for work in unit:
                work()

        # remaining projections are woven into the attention periods; each
        # woven chunk lands (in emission order) before the first scores
        # matmul that reads it.
        def full_unit(halves):
            return lambda: [h() for h in halves]

        woven = [full_unit(proj_unit(wq, qT[0], 0, 1))]
        woven_rest = []
        for nch in range(2, NCH):
            woven_rest.append(full_unit(proj_unit(wq, qT[0], 0, nch)))
        for wt, dst in ((wk, kT[1]), (wq, qT[1])):
            for nch in range(NCH):
                woven_rest.append(full_unit(proj_unit(wt, dst, 1, nch)))

        # V: out[n, c] = xT[:, ntile].T @ wv   -> [128 n, 256 c]
        def v_unit(nt):
            state = {}

            def half(h):
                if h == 0:
                    state["ps"] = spool.tile([P, HL * DH], f32, tag="sp", name="psv")
                ps = state["ps"]
                for kt in range(4 * h, 4 * h + 4):
                    nc.tensor.matmul(
                        ps[:],
                        xT[:, kt, nt * P : (nt + 1) * P],
                        wv[:, kt, :],
                        start=(kt == 0),
                        stop=(kt == KT - 1),
                    )
                if h == 1:
                    # scatter the 4 heads' 64 cols into the [NT, HL, 65] layout
                    nc.vector.tensor_copy(
                        v[:, nt, :, 0:DH],
                        ps[:].rearrange("p (h d) -> p h d", h=HL),
                    )

            return [lambda: half(0), lambda: half(1)]

        v_units = [full_unit(v_unit(nt)) for nt in range(NT)]

        # ---- attention ----
        # 8 blocks of 16 periods (one per (hp, ic)).  ACT runs one
        # [128, 1024] exp per period back-to-back; PE emits scores two
        # periods ahead (spool rotation) plus woven projection work; PV runs
        # as dense 8-matmul bursts every 4 periods (no exp-latency exposure).
        # Block 0 weaves the V projection (PV bursts shifted late until V is
        # ready); blocks 1+ weave the remaining q/k projections.
        blocks = [(hp, ic) for hp in range(2) for ic in range(NCH)]
        ats = {}
        opairs = {}
        sp_ahead = {}

        def emit_scores(b, jb):
            hp, ic = blocks[b]
            i0, j0 = ic * 512, jb * P
            sp = spool.tile([P, 1024], f32, tag="sp", name="sp")
            nc.tensor.matmul(
                sp[:, 0:512],
                kT[hp][0:DH, j0 : j0 + P],
                qT[hp][0:DH, i0 : i0 + 512],
                start=True, stop=True, tile_position=(0, 0),
            )
            nc.tensor.matmul(
                sp[:, 512:1024],
                kT[hp][DH:P, j0 : j0 + P],
                qT[hp][DH:P, i0 : i0 + 512],
                start=True, stop=True, tile_position=(64, 0),
            )
            return sp

        def emit_exp(b, jb, sp):
            at = apool.tile([P, 1024], bf16, tag="at", name="at")
            nc.scalar.activation(at[:], sp[:], mybir.ActivationFunctionType.Exp)
            ats[(b, jb)] = at

        def fetch_scores(b, jb):
            key = (b, jb)
            if key in sp_ahead:
                return sp_ahead.pop(key)
            return emit_scores(b, jb)

        def emit_pv_quarter(b, q):
            """PV matmuls for periods 4q..4q+3 of block b (dense burst)."""
            hp, ic = blocks[b]
            if q == 0:
                opairs[b] = (
                    opool.tile([DH + 1, 512], f32, tag="oA", name="oA"),
                    opool.tile([DH + 1, 512], f32, tag="oB", name="oB"),
                )
            oA, oB = opairs[b]
            for col, o in ((0, oA), (1, oB)):
                for jb in range(4 * q, 4 * q + 4):
                    nc.tensor.matmul(
                        o[:],
                        v[:, jb, 2 * hp + col, :],
                        ats[(b, jb)][:, 512 * col : 512 * col + 512],
                        start=(jb == 0), stop=(jb == NT - 1),
                    )
            for jb in range(4 * q, 4 * q + 4):
                del ats[(b, jb)]
            if q == 3:
                i0 = ic * 512
                os = copool.tile([DH + 1, 2, 512], f32, tag="os", name="os")
                nc.vector.tensor_copy(os[:, 0, :], oA[:])
                nc.vector.tensor_copy(os[:, 1, :], oB[:])
                nc.sync.dma_start(
                    out=out_r[:, 2 * hp : 2 * hp + 2, i0 : i0 + 512],
                    in_=os[:],
                )

        LA = 2  # scores lookahead depth
        nblocks = len(blocks)
        # prime the pipeline, then finish the kT01 projection chunks so the
        # first exp only waits on k01n0 + q01n0
        for nch in range(1, NCH):
            for work in proj_unit(wk, kT[0], 0, nch):
                work()
        for j in range(LA):
            sp_ahead[(0, j)] = emit_scores(0, j)
        for b in range(nblocks):
            for jb in range(NT):
                emit_exp(b, jb, fetch_scores(b, jb))
                la = jb + LA
                if la < NT:
                    if (b, la) not in sp_ahead:
                        sp_ahead[(b, la)] = emit_scores(b, la)
                elif b + 1 < nblocks:
                    sp_ahead[(b + 1, la - NT)] = emit_scores(b + 1, la - NT)
                if jb == NT - 1 and b + 1 < nblocks:
                    # boundary prefetch into the idle third spool slot: gives
                    # ACT a 3rd exp of cover across the 16-matmul PV burst
                    sp_ahead[(b + 1, LA)] = emit_scores(b + 1, LA)
                # woven PE filler
                p = b * NT + jb
                if b == 0:
                    if woven:
                        woven.pop(0)()
                    for _ in range(2):
                        if not woven and v_units:
                            v_units.pop(0)()
                elif woven_rest and (p - NT) % 5 == 4:
                    woven_rest.pop(0)()
                # PV bursts (block 0's deferred until the woven V is ready)
                if b == 0:
                    if jb in (8, 12):
                        emit_pv_quarter(0, (jb - 8) // 4)
                    elif jb == NT - 1:
                        while v_units:
                            v_units.pop(0)()
                        emit_pv_quarter(0, 2)
                        emit_pv_quarter(0, 3)
                elif jb % 4 == 0 and jb > 0:
                    emit_pv_quarter(b, jb // 4 - 1)
                elif jb == NT - 1:
                    emit_pv_quarter(b, 3)

    nc.compile()
    return nc


def _get_nc():
    if "nc" not in _CACHE:
        _CACHE["nc"] = _build_nc()
    return _CACHE["nc"]


def _prepare_in_maps(x, w_qkv):
    bf = ml_dtypes.bfloat16
    x = np.asarray(x, dtype=np.float32)
    w = np.asarray(w_qkv, dtype=np.float32)
    scale = DH ** -0.5
    in_maps = []
    xT_b = [
        np.ascontiguousarray(
            x[b].T.reshape(KT, P, N).transpose(1, 0, 2).reshape(P, KT * N)
        ).astype(bf)
        for b in range(B)
    ]
    for c in range(8):
        b, hg = divmod(c, 4)
        cs = slice(hg * HL * DH, (hg + 1) * HL * DH)
        in_maps.append(
            {
                "xT": xT_b[b],
                "wq": np.ascontiguousarray(w[:, cs] * scale).astype(bf),
                "wk": np.ascontiguousarray(w[:, 1024:2048][:, cs]).astype(bf),
                "wv": np.ascontiguousarray(w[:, 2048:3072][:, cs]).astype(bf),
            }
        )
    return in_maps


def _assemble(outs):
    full = np.empty((B, N, HEADS * DH), dtype=np.float32)
    for c in range(8):
        b, hg = divmod(c, 4)
        o = outs[c].reshape(HL, DH + 1, N)
        norm = o[:, :DH, :] / o[:, DH : DH + 1, :]  # [hl, d, n]
        full[b, :, hg * HL * DH : (hg + 1) * HL * DH] = norm.transpose(2, 0, 1).reshape(
            N, HL * DH
        )
    return full


def kernel(x, w_qkv):
    global LAST_RESULTS
    from concourse.bass_utils import run_bass_kernel_spmd

    nc = _get_nc()
    in_maps = _prepare_in_maps(x, w_qkv)
    last_err = None
    for _ in range(3):  # the runtime occasionally throws a transient device error
        try:
            res = run_bass_kernel_spmd(
                nc,
                in_maps,
                core_ids=list(range(8)),
                trace=TRACE,
                trace_cores=[0] if TRACE else None,
            )
            break
        except Exception as e:
            last_err = e
    else:
        raise last_err
    LAST_RESULTS = res
    return _assemble([r["out"] for r in res.results])



# revision 5
# speedup vs baseline: 1.0475x; 1.0475x over previous
"""Multi-head attention (b=2, n=2048, dim=1024, h=16, dh=64) on 8 TRN2 NeuronCores.

Sharding: 32 (batch, head) pairs -> 8 cores x (1 batch, 4 heads). No collectives.

Per core, per (head-pair hp, 512-query-chunk ic) "block" (8 blocks):
  scores  S^T[j,i] = kT.T @ qT  per (jb, head): two K=64 matmuls row-tiled into a
          [128, 1024] PSUM tile (head A cols 0:512, head B cols 512:1024)
  exp     one instruction per score tile, alternating engines:
            even tiles -> ACT exact exp (f32 PSUM -> bf16 SBUF)
            odd tiles  -> DVE Schraudolph: int16(rint(s*128/ln2 + (127*128-7.5)));
                          the int16 bit pattern IS bf16 exp(s) to ~1.8% rms, which
                          the softmax normalization attenuates well below tolerance
  PV      per (jb): two col-tiled M=64 matmuls (heads A/B) accumulate O^T into one
          shared PSUM bank (partitions 0:64 / 64:128)
  den     per 2 jb: four col-tiled M=1 ones-matmuls re-stream the at slices,
          accumulating per-(head, jb-parity) row sums on PSUM partitions 0/32/64/96
Host divides O^T by (den_even + den_odd) and transposes back to [b, n, h*dh].
"""

import numpy as np
import ml_dtypes

B, N, DIM = 2, 2048, 1024
HEADS, DH = 16, 64
P = 128
KT = DIM // P          # 8 k-tiles
NT = N // P            # 16 j blocks
NCH = N // 512         # 4 n/query chunks
HL = 4                 # local heads per core
NBLK = 8               # (hp, ic) blocks per core
NPAIR = 128            # score tiles / exp tiles per core

LOG2E_SCALE = 184.6650390625        # 128/ln2
SCHRAUD_B = 127.0 * 128.0 - 7.5     # exponent bias - centering correction

_CACHE = {}
LAST_RESULTS = None
TRACE = False


def _build_nc():
    from contextlib import ExitStack

    import concourse.bass as bass
    import concourse.tile as tile
    from concourse import bacc, mybir

    bf16 = mybir.dt.bfloat16
    i16 = mybir.dt.int16
    f32 = mybir.dt.float32
    Exp = mybir.ActivationFunctionType.Exp

    nc = bacc.Bacc("TRN2", target_bir_lowering=False)

    # x^T packed n-chunk-major: element (p, nc, kt, n') = x[b].T[kt*128+p, nc*512+n']
    xT_d = nc.dram_tensor("xT", [P, NCH * KT * 512], bf16, kind="ExternalInput")
    # weights pre-swizzled to the SBUF layout [p, kt, col]
    wq_d = nc.dram_tensor("wq", [P, KT * HL * DH], bf16, kind="ExternalInput")
    wk_d = nc.dram_tensor("wk", [P, KT * HL * DH], bf16, kind="ExternalInput")
    wv_d = nc.dram_tensor("wv", [P, KT * HL * DH], bf16, kind="ExternalInput")
    oo_d = nc.dram_tensor("oo", [NBLK * P, 512], f32, kind="ExternalOutput")
    dd_d = nc.dram_tensor("dd", [NBLK * 4, 512], f32, kind="ExternalOutput")

    xT_r = xT_d[:, :].rearrange("p (c kt n) -> p c kt n", c=NCH, kt=KT)
    wq_r = wq_d[:, :].rearrange("p (kt c) -> p kt c", kt=KT)
    wk_r = wk_d[:, :].rearrange("p (kt c) -> p kt c", kt=KT)
    wv_r = wv_d[:, :].rearrange("p (kt c) -> p kt c", kt=KT)
    oo_r = oo_d[:, :].rearrange("(b p) n -> b p n", b=NBLK)
    dd_r = dd_d[:, :].rearrange("(b g) n -> b g n", b=NBLK)

    with tile.TileContext(nc) as tc, ExitStack() as ctx:
        sing = ctx.enter_context(tc.tile_pool(name="sing", bufs=1))
        spool = ctx.enter_context(
            tc.tile_pool(name="s_ps", bufs=2, space=bass.MemorySpace.PSUM)
        )
        ppool = ctx.enter_context(
            tc.tile_pool(name="p_ps", bufs=2, space=bass.MemorySpace.PSUM)
        )
        opool = ctx.enter_context(
            tc.tile_pool(name="o_ps", bufs=1, space=bass.MemorySpace.PSUM)
        )
        dpool = ctx.enter_context(
            tc.tile_pool(name="d_ps", bufs=1, space=bass.MemorySpace.PSUM)
        )
        apool = ctx.enter_context(tc.tile_pool(name="a_sb", bufs=12))
        copool = ctx.enter_context(tc.tile_pool(name="o_sb", bufs=2))
        cdpool = ctx.enter_context(tc.tile_pool(name="d_sb", bufs=2))

        # persistent SBUF tensors
        xT = sing.tile([P, NCH, KT, 512], bf16, tag="xT")
        wq = sing.tile([P, KT, HL * DH], bf16, tag="wq")
        wk = sing.tile([P, KT, HL * DH], bf16, tag="wk")
        wv = sing.tile([P, KT, HL * DH], bf16, tag="wv")
        qT = [sing.tile([P, N], bf16, tag=f"qT{i}", name=f"qT{i}") for i in range(2)]
        kT = [sing.tile([P, N], bf16, tag=f"kT{i}", name=f"kT{i}") for i in range(2)]
        v = sing.tile([P, NT, HL, DH], bf16, tag="v")
        ones = sing.tile([P, 4], bf16, tag="ones")

        # ---- input DMAs (emitted up front; rings drain asynchronously) ----
        nc.scalar.dma_start(out=wk[:], in_=wk_r[:])
        nc.sync.dma_start(out=xT[:, 0, 0:4, :], in_=xT_r[:, 0, 0:4, :])
        nc.gpsimd.dma_start(out=xT[:, 0, 4:8, :], in_=xT_r[:, 0, 4:8, :])
        nc.scalar.dma_start(out=wq[:], in_=wq_r[:])
        nc.gpsimd.dma_start(out=wv[:], in_=wv_r[:])
        nc.sync.dma_start(out=xT[:, 1, :, :], in_=xT_r[:, 1, :, :])
        nc.sync.dma_start(out=xT[:, 2, :, :], in_=xT_r[:, 2, :, :])
        nc.sync.dma_start(out=xT[:, 3, :, :], in_=xT_r[:, 3, :, :])
        nc.vector.memset(ones[:], 1.0)

        # ---- projection units ----
        def proj_unit(wt, dst, hp, nch):
            def run():
                ps = ppool.tile([P, 512], f32, tag="pp", name="pp")
                for kt in range(KT):
                    nc.tensor.matmul(
                        ps[:],
                        wt[:, kt, hp * P : (hp + 1) * P],
                        xT[:, nch, kt, :],
                        start=(kt == 0),
                        stop=(kt == KT - 1),
                    )
                nc.vector.tensor_copy(dst[:, nch * 512 : (nch + 1) * 512], ps[:])

            return run

        def v_unit(nt):
            def run():
                ps = ppool.tile([P, HL * DH], f32, tag="pp", name="ppv")
                c, sub = nt // 4, nt % 4
                for kt in range(KT):
                    nc.tensor.matmul(
                        ps[:],
                        xT[:, c, kt, sub * P : (sub + 1) * P],
                        wv[:, kt, :],
                        start=(kt == 0),
                        stop=(kt == KT - 1),
                    )
                nc.vector.tensor_copy(
                    v[:, nt, :, :],
                    ps[:].rearrange("p (h d) -> p h d", h=HL),
                )

            return run

        # proj weave schedule: period -> list of units.  v_unit(nt) must be
        # emitted before PV(0, nt); k/q chunks before the score pairs that
        # read them (pair e+1 is emitted at period e).
        sched = {}

        def add(p, u):
            sched.setdefault(p, []).append(u)

        add(0, proj_unit(wk, kT[0], 0, 0))
        add(0, proj_unit(wq, qT[0], 0, 0))
        add(1, v_unit(0))
        add(2, proj_unit(wk, kT[0], 0, 1))
        add(2, v_unit(1))
        for i in range(2, 5):
            add(3 + (i - 2), v_unit(i))
        add(6, proj_unit(wk, kT[0], 0, 2))
        for i in range(5, 10):
            add(6 + (i - 5), v_unit(i))
        add(10, proj_unit(wk, kT[0], 0, 3))
        for i in range(10, 16):
            add(11 + (i - 10), v_unit(i))
        add(14, proj_unit(wq, qT[0], 0, 1))
        add(29, proj_unit(wq, qT[0], 0, 2))
        add(45, proj_unit(wq, qT[0], 0, 3))
        add(56, proj_unit(wk, kT[1], 1, 0))
        add(59, proj_unit(wq, qT[1], 1, 0))
        add(62, proj_unit(wk, kT[1], 1, 1))
        add(66, proj_unit(wk, kT[1], 1, 2))
        add(70, proj_unit(wk, kT[1], 1, 3))
        add(77, proj_unit(wq, qT[1], 1, 1))
        add(93, proj_unit(wq, qT[1], 1, 2))
        add(109, proj_unit(wq, qT[1], 1, 3))

        # ---- attention ----
        # pair/tile e <-> (block b = e//16, jb = e%16); block b = (hp=b//4, ic=b%4)
        sp_tiles = {}
        at_tiles = {}

        def emit_scores(e):
            b, jb = e // 16, e % 16
            hp, ic = b // 4, b % 4
            i0, j0 = ic * 512, jb * P
            sp = spool.tile([P, 1024], f32, tag="sp", name="sp")
            nc.tensor.matmul(
                sp[:, 0:512],
                kT[hp][0:DH, j0 : j0 + P],
                qT[hp][0:DH, i0 : i0 + 512],
                start=True, stop=True, tile_position=(0, 0),
            )
            nc.tensor.matmul(
                sp[:, 512:1024],
                kT[hp][DH:P, j0 : j0 + P],
                qT[hp][DH:P, i0 : i0 + 512],
                start=True, stop=True, tile_position=(64, 0),
            )
            sp_tiles[e] = sp

        def emit_exp(e):
            sp = sp_tiles.pop(e)
            if e % 2 == 0:
                at = apool.tile([P, 1024], bf16, tag="at", name="at")
                nc.scalar.activation(at[:], sp[:], Exp)
                at_tiles[e] = at[:]
            else:
                at = apool.tile([P, 1024], i16, tag="at", name="atd")
                nc.vector.tensor_scalar(
                    out=at[:],
                    in0=sp[:],
                    scalar1=LOG2E_SCALE,
                    scalar2=SCHRAUD_B,
                    op0=mybir.AluOpType.mult,
                    op1=mybir.AluOpType.add,
                )
                at_tiles[e] = at[:].bitcast(bf16)

        # PV / den pass work queue, in block order
        ostate = {}

        def pv_pass(b, jb):
            hp = b // 4
            if jb == 0:
                ostate[b] = (
                    opool.tile([P, 512], f32, tag="oo", name="oo"),
                    dpool.tile([97, 512], f32, tag="dd", name="dd"),
                )
            o_ps, _ = ostate[b]
            at = at_tiles[16 * b + jb]
            for col in range(2):
                nc.tensor.matmul(
                    o_ps[64 * col : 64 * col + 64, :],
                    v[:, jb, 2 * hp + col, :],
                    at[:, 512 * col : 512 * col + 512],
                    start=(jb == 0), stop=(jb == NT - 1),
                    tile_position=(0, 64 * col),
                )

        def den_pass(b, s):
            _, den_ps = ostate[b]
            a0 = at_tiles[16 * b + 2 * s]
            a1 = at_tiles[16 * b + 2 * s + 1]
            for g, (a, off) in enumerate(((a0, 0), (a1, 0), (a0, 512), (a1, 512))):
                nc.tensor.matmul(
                    den_ps[32 * g : 32 * g + 1, :],
                    ones[:, g : g + 1],
                    a[:, off : off + 512],
                    start=(s == 0), stop=(s == 7),
                    tile_position=(0, 32 * g),
                )

        def finish_block(b):
            o_ps, den_ps = ostate.pop(b)
            os = copool.tile([P, 512], f32, tag="os", name="os")
            ds = cdpool.tile([97, 512], f32, tag="ds", name="ds")
            nc.scalar.copy(os[:], o_ps[:])
            nc.scalar.copy(ds[:], den_ps[:])
            nc.gpsimd.dma_start(out=oo_r[b], in_=os[:])
            nc.gpsimd.dma_start(out=dd_r[b], in_=ds[0:97:32, :])
            # release at tiles of this block
            for jb in range(NT):
                del at_tiles[16 * b + jb]

        # pass queue: (kind, args, max_at_tile_needed, min_period)
        passes = []
        for b in range(NBLK):
            for s in range(8):
                jb0, jb1 = 2 * s, 2 * s + 1
                vgate0 = jb0 + 2 if b == 0 else 0
                vgate1 = jb1 + 2 if b == 0 else 0
                passes.append(("pv", (b, jb0), 16 * b + jb0, vgate0))
                passes.append(("pv", (b, jb1), 16 * b + jb1, vgate1))
                passes.append(("den", (b, s), 16 * b + jb1, vgate1))
            passes.append(("fin", (b,), 16 * b + 15, 0))
        passes.reverse()  # pop from the end

        # ---- main period loop ----
        for e in range(NPAIR + 8):
            for u in sched.pop(e, ()):
                u()
            if e == 0:
                emit_scores(0)
            if e + 1 < NPAIR:
                emit_scores(e + 1)
            if e < NPAIR:
                emit_exp(e)
            budget = 3
            while passes and budget > 0:
                kind, args, need_tile, min_p = passes[-1]
                if need_tile > e - 1 or min_p > e:
                    break
                passes.pop()
                if kind == "pv":
                    pv_pass(*args)
                    budget -= 1
                elif kind == "den":
                    den_pass(*args)
                    budget -= 1
                else:
                    finish_block(*args)
        assert not passes, f"{len(passes)} passes left unemitted"
        assert not sched, f"unconsumed proj units at periods {sorted(sched)}"

    nc.compile()
    return nc


def _get_nc():
    if "nc" not in _CACHE:
        _CACHE["nc"] = _build_nc()
    return _CACHE["nc"]


def _prepare_in_maps(x, w_qkv):
    bf = ml_dtypes.bfloat16
    x = np.asarray(x, dtype=np.float32)
    w = np.asarray(w_qkv, dtype=np.float32)
    scale = DH ** -0.5
    in_maps = []
    # x[b].T [dim, n] -> [kt, p, nc, n'] -> (p, nc, kt, n') packed
    xT_b = [
        np.ascontiguousarray(
            x[b].T.reshape(KT, P, NCH, 512).transpose(1, 2, 0, 3).reshape(P, NCH * KT * 512)
        ).astype(bf)
        for b in range(B)
    ]

    def swz(wcols):  # [1024, 256] -> [p, kt*cols] swizzled
        return np.ascontiguousarray(
            wcols.reshape(KT, P, HL * DH).transpose(1, 0, 2).reshape(P, KT * HL * DH)
        ).astype(bf)

    for c in range(8):
        b, hg = divmod(c, 4)
        cs = slice(hg * HL * DH, (hg + 1) * HL * DH)
        in_maps.append(
            {
                "xT": xT_b[b],
                "wq": swz(w[:, 0:1024][:, cs] * scale),
                "wk": swz(w[:, 1024:2048][:, cs]),
                "wv": swz(w[:, 2048:3072][:, cs]),
            }
        )
    return in_maps


def _assemble(outs):
    full = np.empty((B, N, HEADS * DH), dtype=np.float32)
    for c in range(8):
        b, hg = divmod(c, 4)
        oo = outs[c]["oo"].reshape(NBLK, P, 512)
        dd = outs[c]["dd"].reshape(NBLK, 4, 512)
        for blk in range(NBLK):
            hp, ic = blk // 4, blk % 4
            for h01 in range(2):
                den = dd[blk, 2 * h01] + dd[blk, 2 * h01 + 1]
                o = oo[blk, 64 * h01 : 64 * h01 + 64, :] / den  # [64, 512]
                col = (hg * HL + 2 * hp + h01) * DH
                full[b, ic * 512 : (ic + 1) * 512, col : col + DH] = o.T
    return full


def kernel(x, w_qkv):
    global LAST_RESULTS
    from concourse.bass_utils import run_bass_kernel_spmd

    nc = _get_nc()
    in_maps = _prepare_in_maps(x, w_qkv)
    last_err = None
    for _ in range(3):  # the runtime occasionally throws a transient device error
        try:
            res = run_bass_kernel_spmd(
                nc,
                in_maps,
                core_ids=list(range(8)),
                trace=TRACE,
                trace_cores=[0] if TRACE else None,
            )
            break
        except Exception as e:
            last_err = e
    else:
        raise last_err
    LAST_RESULTS = res
    return _assemble([r for r in res.results])
